# revision 1
# baseline (speedup 1.0000x reference)
"""Trainium2 Bass kernel for the DGN message-passing network.

Computation (per batch item b):
    h = relu(x @ enc_w + enc_b)                      [N, H]
    for p in 0..P-1:
        v = relu(h @ wv[p] + bv[p]); q = relu(h @ wq[p] + bq[p]); k = relu(h @ wk[p] + bk[p])
        att = softmax(q @ k.T  masked by mask, axis=-1)
        h = relu((att @ v) @ wo[p] + bo[p])
    y = h @ qw + qb                                  [N, A]

Sharding: data-parallel over the batch dim across 8 NeuronCores (16 items
per core), weights replicated, no cross-core communication.

On-chip layout: everything is kept transposed ([H, N] with H on partitions)
so no transposes are ever required:
  * hT/qT/kT = [H=128, N=512];   projections:  xT = wq.T @ hT  (lhsT = wq)
  * scoresT[m, n] = q[n]·k[m] computed directly as kT-chunk.T @ qT
  * softmax over m (= partition axis of scoresT) is done as
    exp(s)*mask -> rowsum via an all-ones [128,128] matmul (which lands the
    row-sum broadcast across all partitions) -> multiply by reciprocal.
    No max-subtraction: scores of this model are O(5), exp is safe, and
    softmax is shift-invariant so the result matches the reference.
  * v is needed m-on-partitions for the att@v contraction, so it is built
    natively as 4 row chunks [128, H]; wv is duplicated to [H, 256] so the
    float32r matmul stays on its fast path (moving dim >= 256).
"""

import numpy as np

import concourse.bass as bass
import concourse.mybir as mybir
import concourse.tile as tile
from concourse.bass import ts
from concourse.bass_utils import run_bass_kernel_spmd

F32 = mybir.dt.float32
F32R = mybir.dt.float32r
BF16 = mybir.dt.bfloat16
AF = mybir.ActivationFunctionType
OP = mybir.AluOpType

DEBUG_DUMP = False  # set True before first build to dump item-0 intermediates

N_CORES = 8
B, N, DIN, H, P, A = 128, 512, 64, 128, 2, 16
IPC = B // N_CORES  # batch items per core
NCH = N // 128      # 128-row chunks of the agent dim


def _spill_excess_waits(nc):
    """Walrus codegen has limited sync-wait slots per instruction: a
    self-loading fp32/fp32r Matmult takes only 1 (waits land on its fused
    LDWEIGHTS micro-op) and sequencer ctrl ops (Drain/NoOp) take 4. Spill
    excess waits onto NoOps inserted just before the instruction on the same
    engine - the engine blocks at the NoOp, so ordering semantics are kept.
    """
    counter = [0]

    def make_nop(engine, waits):
        counter[0] += 1
        nop = mybir.InstNoOp(name=f"I-waitspill-{counter[0]}")
        nop.engine = engine
        nop.sync_info = mybir.SyncInfo(on_wait=list(waits), on_update=[])
        return nop

    def sem_clear_insts(inst):
        """This walrus build rejects EVENT_SEMAPHORE_RANGE_CLEAR ("ISA wrong
        length"); expand Tile's tail range-clear into per-sem writes."""
        first = inst.ant_dict["range_first"]
        last = inst.ant_dict["range_last"]
        res = []
        for s in range(first, last + 1):
            counter[0] += 1
            ev = mybir.InstEventSemaphore(name=f"I-semclear-{counter[0]}")
            ev.engine = inst.engine
            ev.sync_info = mybir.SyncInfo(
                on_wait=list(inst.sync_info.on_wait) if (s == first and inst.sync_info) else [],
                on_update=[mybir.SyncUpdate(
                    sync_type="semaphore", id=s,
                    update_mode="sem-wr-imm", update_value=0,
                )],
            )
            res.append(ev)
        return res

    for fn in nc.m.functions:
        for blk in fn.blocks:
            out = []
            for inst in blk.instructions:
                if (type(inst).__name__ == "InstISA"
                        and inst.ant_dict.get("header", {}).get("opcode") == 176):
                    out.extend(sem_clear_insts(inst))
                    continue
                si = inst.sync_info
                waits = list(si.on_wait) if si is not None else []
                limit = 1
                if len(waits) > limit:
                    keep = waits[-limit:] if limit else []
                    spill = waits[: len(waits) - limit]
                    for w in spill:
                        out.append(make_nop(inst.engine, [w]))
                    inst.sync_info.on_wait = keep
                out.append(inst)
            blk.instructions = out


def build_program():
    nc = bass.Bass("TRN2", target_bir_lowering=False, debug=False)

    xt_d = nc.dram_tensor("xt", [IPC, DIN, N], BF16, kind="ExternalInput").ap()
    mt_d = nc.dram_tensor("maskt", [IPC, N, N], BF16, kind="ExternalInput").ap()
    encw_d = nc.dram_tensor("enc_w", [DIN, H], BF16, kind="ExternalInput").ap()
    encb_d = nc.dram_tensor("enc_b", [H, 1], F32, kind="ExternalInput").ap()
    wq_d = nc.dram_tensor("wq", [P, H, H], BF16, kind="ExternalInput").ap()
    wk_d = nc.dram_tensor("wk", [P, H, H], BF16, kind="ExternalInput").ap()
    wv2_d = nc.dram_tensor("wv2", [P, H, 2 * H], BF16, kind="ExternalInput").ap()
    wo_d = nc.dram_tensor("wo", [P, H, H], BF16, kind="ExternalInput").ap()
    bq_d = nc.dram_tensor("bq", [P, H, 1], F32, kind="ExternalInput").ap()
    bk_d = nc.dram_tensor("bk", [P, H, 1], F32, kind="ExternalInput").ap()
    bv_d = nc.dram_tensor("bv", [P, H, 1], F32, kind="ExternalInput").ap()
    bvr_d = nc.dram_tensor("bvr", [P, 1, 2 * H], BF16, kind="ExternalInput").ap()
    bo_d = nc.dram_tensor("bo", [P, H, 1], F32, kind="ExternalInput").ap()
    qw_d = nc.dram_tensor("qw", [H, A], BF16, kind="ExternalInput").ap()
    ones_d = nc.dram_tensor("ones", [128, 128], BF16, kind="ExternalInput").ap()
    qb_d = nc.dram_tensor("qb", [A, 1], F32, kind="ExternalInput").ap()
    yt_d = nc.dram_tensor("yt", [IPC, A, N], F32, kind="ExternalOutput").ap()
    dbg = {}
    if DEBUG_DUMP:
        for nm, shape in [("dbg_h", [H, N]), ("dbg_q", [H, N]), ("dbg_k", [H, N]),
                          ("dbg_v", [128, NCH * H]), ("dbg_p", [128, NCH * N]),
                          ("dbg_r", [H, N]), ("dbg_o", [H, N]), ("dbg_h2", [H, N])]:
            dbg[nm] = nc.dram_tensor(nm, shape, F32, kind="ExternalOutput").ap()

    with tile.TileContext(nc) as tc:
        with (
            tc.tile_pool(name="weights", bufs=1) as wpool,
            tc.tile_pool(name="xin", bufs=2) as xpool,
            tc.tile_pool(name="maskin", bufs=2) as mpool,
            tc.tile_pool(name="hbuf", bufs=3) as hpool,
            tc.tile_pool(name="qbuf", bufs=2) as qpool,
            tc.tile_pool(name="kbuf", bufs=2) as kpool,
            tc.tile_pool(name="vbuf", bufs=2) as vpool,
            tc.tile_pool(name="pbuf", bufs=2) as ppool,
            tc.tile_pool(name="rbuf", bufs=2) as rpool,
            tc.tile_pool(name="obuf", bufs=2) as opool,
            tc.tile_pool(name="ybuf", bufs=2) as ypool,
            tc.tile_pool(name="ppsum", bufs=2, space="PSUM") as ppsum,
            tc.tile_pool(name="spsum", bufs=2, space="PSUM") as spsum,
            tc.tile_pool(name="rpsum", bufs=1, space="PSUM") as rpsum,
            tc.tile_pool(name="opsum", bufs=1, space="PSUM") as opsum,
        ):
            # ---- resident weights ----
            encw_t = wpool.tile([DIN, H], BF16, tag="encw")
            nc.sync.dma_start(out=encw_t[:], in_=encw_d[:])
            encb_t = wpool.tile([H, 1], F32, tag="encb")
            nc.sync.dma_start(out=encb_t[:], in_=encb_d[:])
            qw_t = wpool.tile([H, A], BF16, tag="qw")
            nc.sync.dma_start(out=qw_t[:], in_=qw_d[:])
            qb_t = wpool.tile([A, 1], F32, tag="qb")
            nc.sync.dma_start(out=qb_t[:], in_=qb_d[:])
            ones_t = wpool.tile([128, 128], BF16, tag="ones")
            nc.sync.dma_start(out=ones_t[:], in_=ones_d[:])

            wq_t, wk_t, wv2_t, wo_t, bq_t, bk_t, bv_t, bo_t = [], [], [], [], [], [], [], []
            bvr_t = []
            for p in range(P):
                t = wpool.tile([1, 2 * H], BF16, tag=f"bvr{p}")
                nc.sync.dma_start(out=t[:], in_=bvr_d[p])
                bvr_t.append(t)
            for p in range(P):
                for lst, dram, shape, tag in (
                    (wq_t, wq_d, [H, H], "wq"),
                    (wk_t, wk_d, [H, H], "wk"),
                    (wv2_t, wv2_d, [H, 2 * H], "wv2"),
                    (wo_t, wo_d, [H, H], "wo"),
                    (bq_t, bq_d, [H, 1], "bq"),
                    (bk_t, bk_d, [H, 1], "bk"),
                    (bv_t, bv_d, [H, 1], "bv"),
                    (bo_t, bo_d, [H, 1], "bo"),
                ):
                    wdt = BF16 if tag in ("wq", "wk", "wv2", "wo") else F32
                    t = wpool.tile(shape, wdt, tag=f"{tag}{p}")
                    nc.sync.dma_start(out=t[:], in_=dram[p])
                    lst.append(t)

            # ---- per-item pipeline ----
            for i in range(IPC):
                xt_t = xpool.tile([DIN, N], BF16, tag="xt")
                nc.sync.dma_start(out=xt_t[:], in_=xt_d[i])
                mt_t = mpool.tile([128, NCH * N], BF16, tag="mt")
                for c in range(NCH):
                    nc.sync.dma_start(out=mt_t[:, ts(c, N)], in_=mt_d[i, ts(c, 128)])

                # encoder: hT = relu(enc_w.T @ xT + enc_b)
                hp = ppsum.tile([H, N], F32, tag="proj")
                nc.tensor.matmul(hp[:], lhsT=(encw_t[:]), rhs=(xt_t[:]), start=True, stop=True)
                hT = hpool.tile([H, N], BF16, tag="h")
                nc.scalar.activation(hT[:], hp[:], AF.Relu, bias=encb_t[:])
                if DEBUG_DUMP and i == 0:
                    nc.sync.dma_start(out=dbg["dbg_h"][:], in_=hT[:])

                for p in range(P):
                    # projections
                    qp = ppsum.tile([H, N], F32, tag="proj")
                    nc.tensor.matmul(qp[:], lhsT=(wq_t[p][:]), rhs=(hT[:]), start=True, stop=True)
                    qT = qpool.tile([H, N], BF16, tag="q")
                    nc.vector.tensor_scalar(
                        out=qT[:], in0=qp[:], scalar1=bq_t[p][:], scalar2=0.0,
                        op0=OP.add, op1=OP.max,
                    )

                    kp = ppsum.tile([H, N], F32, tag="proj")
                    nc.tensor.matmul(kp[:], lhsT=(wk_t[p][:]), rhs=(hT[:]), start=True, stop=True)
                    kT = kpool.tile([H, N], BF16, tag="k")
                    nc.vector.tensor_scalar(
                        out=kT[:], in0=kp[:], scalar1=bk_t[p][:], scalar2=0.0,
                        op0=OP.add, op1=OP.max,
                    )

                    # v in natural [m, h] layout, 4 row chunks
                    vn = vpool.tile([128, NCH * H], BF16, tag="v")
                    for c in range(NCH):
                        vp = ppsum.tile([128, 2 * H], F32, tag="proj")
                        # vn is in natural [m, h] layout, so bv must be added
                        # along the FREE axis: preload ones(x)bv into PSUM via
                        # a K=1 matmul, then accumulate h @ wv2 on top.
                        nc.tensor.matmul(
                            vp[:], lhsT=(ones_t[0:1, :]), rhs=(bvr_t[p][:]),
                            start=True, stop=False,
                        )
                        nc.tensor.matmul(
                            vp[:], lhsT=(hT[:, ts(c, 128)]), rhs=(wv2_t[p][:]),
                            start=False, stop=True,
                        )
                        nc.vector.tensor_scalar_max(vn[:, ts(c, H)], vp[:, :H], 0.0)

                    if DEBUG_DUMP and i == 0 and p == 0:
                        nc.sync.dma_start(out=dbg["dbg_q"][:], in_=qT[:])
                        nc.sync.dma_start(out=dbg["dbg_k"][:], in_=kT[:])
                        nc.sync.dma_start(out=dbg["dbg_v"][:], in_=vn[:])
                    # scoresT[m, n]: two double-buffered [128,1024] PSUM
                    # pair-tiles so the next pair's matmuls overlap this
                    # pair's exp; exp is one ACTIVATE per pair.
                    pT = ppool.tile([128, NCH * N], BF16, tag="p")
                    for j in range(NCH // 2):
                        scp = spsum.tile([128, 2 * N], F32, tag="sc")
                        for cc in range(2):
                            c = 2 * j + cc
                            nc.tensor.matmul(
                                scp[:, ts(cc, N)], lhsT=(kT[:, ts(c, 128)]), rhs=(qT[:]),
                                start=True, stop=True,
                            )
                        nc.scalar.activation(pT[:, ts(j, 2 * N)], scp[:], AF.Exp)
                    # mask multiply (elementwise), split DVE / GpSimd
                    for c in range(NCH):
                        eng = nc.vector if c % 2 == 0 else nc.gpsimd
                        eng.tensor_tensor(
                            out=pT[:, ts(c, N)], in0=pT[:, ts(c, N)],
                            in1=mt_t[:, ts(c, N)], op=OP.mult,
                        )

                    if DEBUG_DUMP and i == 0 and p == 0:
                        nc.sync.dma_start(out=dbg["dbg_p"][:], in_=pT[:])
                    # rowsum over m via all-ones matmul (result broadcast on partitions)
                    rs = rpsum.tile([128, N], F32, tag="rs")
                    for c in range(NCH):
                        nc.tensor.matmul(
                            rs[:], lhsT=(ones_t[:]), rhs=(pT[:, ts(c, N)]),
                            start=(c == 0), stop=(c == NCH - 1),
                        )
                    # outT = v.T @ p  (accumulate over m chunks)
                    ot = opsum.tile([H, N], F32, tag="ot")
                    for c in range(NCH):
                        nc.tensor.matmul(
                            ot[:], lhsT=(vn[:, ts(c, H)]), rhs=(pT[:, ts(c, N)]),
                            start=(c == 0), stop=(c == NCH - 1),
                        )
                    # 1/rowsum = exp(-ln(rowsum)): Ln and Exp live in the
                    # same ACT table set as the softmax Exp, so no table
                    # switching (Reciprocal would thrash 2.7us loads).
                    lnr = rpool.tile([H, N], F32, tag="lnr")
                    nc.scalar.activation(lnr[:], rs[:], AF.Ln)
                    recipb = rpool.tile([H, N], F32, tag="recip")
                    nc.scalar.activation(recipb[:], lnr[:], AF.Exp, scale=-1.0)
                    otn = opool.tile([H, N], BF16, tag="otn")
                    nc.vector.tensor_tensor(out=otn[:], in0=ot[:], in1=recipb[:], op=OP.mult)
                    if DEBUG_DUMP and i == 0 and p == 0:
                        nc.sync.dma_start(out=dbg["dbg_r"][:], in_=recipb[:])
                        nc.sync.dma_start(out=dbg["dbg_o"][:], in_=otn[:])

                    # out projection -> new hT
                    h2p = ppsum.tile([H, N], F32, tag="proj")
                    nc.tensor.matmul(h2p[:], lhsT=(wo_t[p][:]), rhs=(otn[:]), start=True, stop=True)
                    hT = hpool.tile([H, N], BF16, tag="h")
                    nc.vector.tensor_scalar(
                        out=hT[:], in0=h2p[:], scalar1=bo_t[p][:], scalar2=0.0,
                        op0=OP.add, op1=OP.max,
                    )
                    if DEBUG_DUMP and i == 0 and p == 0:
                        nc.sync.dma_start(out=dbg["dbg_h2"][:], in_=hT[:])

                # Q head: yT = qw.T @ hT + qb
                yp = ppsum.tile([A, N], F32, tag="proj")
                nc.tensor.matmul(yp[:], lhsT=(qw_t[:]), rhs=(hT[:]), start=True, stop=True)
                yt_t = ypool.tile([A, N], F32, tag="y")
                nc.vector.tensor_scalar_add(yt_t[:], yp[:], qb_t[:])
                nc.sync.dma_start(out=yt_d[i], in_=yt_t[:])

    _spill_excess_waits(nc)
    return nc


_prog_cache = None


def _get_program():
    global _prog_cache
    if _prog_cache is None:
        _prog_cache = build_program()
    return _prog_cache


def _make_in_maps(x, mask, enc_w, enc_b, wv, bv, wk, bk, wq, bq, wo, bo, qw, qb):
    import ml_dtypes
    bf = lambda a: np.ascontiguousarray(np.asarray(a, dtype=np.float32).astype(ml_dtypes.bfloat16))
    f = lambda a: np.ascontiguousarray(np.asarray(a, dtype=np.float32))
    x, mask = f(x), f(mask)
    shared = {
        "enc_w": bf(enc_w),
        "enc_b": f(enc_b).reshape(H, 1),
        "wq": bf(wq),
        "wk": bf(wk),
        "wv2": np.ascontiguousarray(np.concatenate([bf(wv), bf(wv)], axis=2)),
        "wo": bf(wo),
        "bq": f(bq).reshape(P, H, 1),
        "bk": f(bk).reshape(P, H, 1),
        "bv": f(bv).reshape(P, H, 1),
        "bvr": np.ascontiguousarray(np.concatenate([bf(bv), bf(bv)], axis=1).reshape(P, 1, 2 * H)),
        "bo": f(bo).reshape(P, H, 1),
        "qw": bf(qw),
        "ones": np.ones((128, 128), dtype=ml_dtypes.bfloat16),
        "qb": f(qb).reshape(A, 1),
    }
    in_maps = []
    for c in range(N_CORES):
        sl = slice(c * IPC, (c + 1) * IPC)
        in_maps.append({
            "xt": np.ascontiguousarray(x[sl].transpose(0, 2, 1).astype(ml_dtypes.bfloat16)),
            "maskt": np.ascontiguousarray(mask[sl].transpose(0, 2, 1).astype(ml_dtypes.bfloat16)),
            **shared,
        })
    return in_maps


def run(trace=False, **inputs):
    nc = _get_program()
    in_maps = _make_in_maps(**inputs)
    res = run_bass_kernel_spmd(nc, in_maps, list(range(N_CORES)), trace=trace)
    y = np.concatenate(
        [r["yt"].transpose(0, 2, 1) for r in res.results], axis=0
    ).astype(np.float32)
    return y, res


def kernel(**inputs):
    y, _ = run(trace=False, **inputs)
    return y



# revision 2
# speedup vs baseline: 1.0943x; 1.0943x over previous
"""Trainium2 Bass kernel for the DGN message-passing network.

Computation (per batch item b):
    h = relu(x @ enc_w + enc_b)                      [N, H]
    for p in 0..P-1:
        v = relu(h @ wv[p] + bv[p]); q = relu(h @ wq[p] + bq[p]); k = relu(h @ wk[p] + bk[p])
        att = softmax(q @ k.T  masked by mask, axis=-1)
        h = relu((att @ v) @ wo[p] + bo[p])
    y = h @ qw + qb                                  [N, A]

Sharding: data-parallel over the batch dim across 8 NeuronCores (16 items
per core), weights replicated, no cross-core communication.

On-chip layout: everything is kept transposed ([H, N] with H on partitions)
so no transposes are ever required:
  * hT/qT/kT = [H=128, N=512];   projections:  xT = wq.T @ hT  (lhsT = wq)
  * scoresT[m, n] = q[n]·k[m] computed directly as kT-chunk.T @ qT
  * softmax over m (= partition axis of scoresT) is done as
    exp(s)*mask -> rowsum via an all-ones [128,128] matmul (which lands the
    row-sum broadcast across all partitions) -> multiply by reciprocal.
    No max-subtraction: scores of this model are O(8), exp is safe, and
    softmax is shift-invariant so the result matches the reference.
  * v is needed m-on-partitions for the att@v contraction, so it is built
    natively as 4 row chunks packed in one [128, 4*H] PSUM tile; the bias
    (which varies along the free axis there) is preloaded with a single
    K=1 ones x bv4 matmul, then the 4 h-chunk matmuls accumulate on top.

Engine budget per pass-unit (16 items x 2 passes), targeting ~3.7us/unit
on every engine so the PE never starves (HAM stays warm at 2.4 GHz):
  PE : q,k MMs + v preload/4MM + 4 score MMs + 4 rowsum + 4 attv + out MM
  ACT: exp x2 (wide [128,1024]) + ln + exp(-ln) + q-relu (+ enc relu)
  DVE: k-relu, v-relu, h2-relu, otn mult, 2 mask mults (+ y bias-add)
  GPS: 2 mask mults
"""

import numpy as np

import concourse.bass as bass
import concourse.mybir as mybir
import concourse.tile as tile
from concourse.bass import ts
from concourse.bass_utils import run_bass_kernel_spmd

F32 = mybir.dt.float32
BF16 = mybir.dt.bfloat16
AF = mybir.ActivationFunctionType
OP = mybir.AluOpType

N_CORES = 8
B, N, DIN, H, P, A = 128, 512, 64, 128, 2, 16
IPC = B // N_CORES  # batch items per core
NCH = N // 128      # 128-row chunks of the agent dim


def _spill_excess_waits(nc):
    """Walrus codegen has limited sync-wait slots per instruction: a
    self-loading fp32/fp32r Matmult takes only 1 (waits land on its fused
    LDWEIGHTS micro-op) and sequencer ctrl ops (Drain/NoOp) take 4. Spill
    excess waits onto NoOps inserted just before the instruction on the same
    engine - the engine blocks at the NoOp, so ordering semantics are kept.
    """
    counter = [0]

    def make_nop(engine, waits):
        counter[0] += 1
        nop = mybir.InstNoOp(name=f"I-waitspill-{counter[0]}")
        nop.engine = engine
        nop.sync_info = mybir.SyncInfo(on_wait=list(waits), on_update=[])
        return nop

    def sem_clear_insts(inst):
        """This walrus build rejects EVENT_SEMAPHORE_RANGE_CLEAR ("ISA wrong
        length"); expand Tile's tail range-clear into per-sem writes."""
        first = inst.ant_dict["range_first"]
        last = inst.ant_dict["range_last"]
        res = []
        for s in range(first, last + 1):
            counter[0] += 1
            ev = mybir.InstEventSemaphore(name=f"I-semclear-{counter[0]}")
            ev.engine = inst.engine
            ev.sync_info = mybir.SyncInfo(
                on_wait=list(inst.sync_info.on_wait) if (s == first and inst.sync_info) else [],
                on_update=[mybir.SyncUpdate(
                    sync_type="semaphore", id=s,
                    update_mode="sem-wr-imm", update_value=0,
                )],
            )
            res.append(ev)
        return res

    for fn in nc.m.functions:
        for blk in fn.blocks:
            out = []
            for inst in blk.instructions:
                if (type(inst).__name__ == "InstISA"
                        and inst.ant_dict.get("header", {}).get("opcode") == 176):
                    out.extend(sem_clear_insts(inst))
                    continue
                si = inst.sync_info
                waits = list(si.on_wait) if si is not None else []
                limit = 1
                if len(waits) > limit:
                    keep = waits[-limit:] if limit else []
                    spill = waits[: len(waits) - limit]
                    for w in spill:
                        out.append(make_nop(inst.engine, [w]))
                    inst.sync_info.on_wait = keep
                out.append(inst)
            blk.instructions = out


def build_program():
    nc = bass.Bass("TRN2", target_bir_lowering=False, debug=False)

    xt_d = nc.dram_tensor("xt", [IPC, DIN, N], BF16, kind="ExternalInput").ap()
    mt_d = nc.dram_tensor("maskt", [IPC, N, N], BF16, kind="ExternalInput").ap()
    encw_d = nc.dram_tensor("enc_w", [DIN, H], BF16, kind="ExternalInput").ap()
    encb_d = nc.dram_tensor("enc_b", [H, 1], F32, kind="ExternalInput").ap()
    wq_d = nc.dram_tensor("wq", [P, H, H], BF16, kind="ExternalInput").ap()
    wk_d = nc.dram_tensor("wk", [P, H, H], BF16, kind="ExternalInput").ap()
    wv_d = nc.dram_tensor("wv", [P, H, H], BF16, kind="ExternalInput").ap()
    wo_d = nc.dram_tensor("wo", [P, H, H], BF16, kind="ExternalInput").ap()
    bq_d = nc.dram_tensor("bq", [P, H, 1], F32, kind="ExternalInput").ap()
    bk_d = nc.dram_tensor("bk", [P, H, 1], F32, kind="ExternalInput").ap()
    bv4_d = nc.dram_tensor("bv4", [P, 1, NCH * H], BF16, kind="ExternalInput").ap()
    bo_d = nc.dram_tensor("bo", [P, H, 1], F32, kind="ExternalInput").ap()
    qw_d = nc.dram_tensor("qw", [H, A], BF16, kind="ExternalInput").ap()
    ones_d = nc.dram_tensor("ones", [128, 128], BF16, kind="ExternalInput").ap()
    qb_d = nc.dram_tensor("qb", [A, 1], F32, kind="ExternalInput").ap()
    yt_d = nc.dram_tensor("yt", [IPC, A, N], F32, kind="ExternalOutput").ap()

    with tile.TileContext(nc) as tc:
        with (
            tc.tile_pool(name="weights", bufs=1) as wpool,
            tc.tile_pool(name="xin", bufs=3) as xpool,
            tc.tile_pool(name="maskin", bufs=3) as mpool,
            tc.tile_pool(name="hbuf", bufs=5) as hpool,
            tc.tile_pool(name="qbuf", bufs=2) as qpool,
            tc.tile_pool(name="kbuf", bufs=2) as kpool,
            tc.tile_pool(name="vbuf", bufs=2) as vpool,
            tc.tile_pool(name="pbuf", bufs=3) as ppool,
            tc.tile_pool(name="rbuf", bufs=2) as rpool,
            tc.tile_pool(name="obuf", bufs=2) as opool,
            tc.tile_pool(name="ybuf", bufs=2) as ypool,
            tc.tile_pool(name="ppsum", bufs=2, space="PSUM") as ppsum,
            tc.tile_pool(name="spsum", bufs=2, space="PSUM") as spsum,
            tc.tile_pool(name="rpsum", bufs=1, space="PSUM") as rpsum,
            tc.tile_pool(name="opsum", bufs=1, space="PSUM") as opsum,
        ):
            # ---- resident weights ----
            encw_t = wpool.tile([DIN, H], BF16, tag="encw")
            nc.sync.dma_start(out=encw_t[:], in_=encw_d[:])
            encb_t = wpool.tile([H, 1], F32, tag="encb")
            nc.sync.dma_start(out=encb_t[:], in_=encb_d[:])
            qw_t = wpool.tile([H, A], BF16, tag="qw")
            nc.sync.dma_start(out=qw_t[:], in_=qw_d[:])
            qb_t = wpool.tile([A, 1], F32, tag="qb")
            nc.sync.dma_start(out=qb_t[:], in_=qb_d[:])
            ones_t = wpool.tile([128, 128], BF16, tag="ones")
            nc.sync.dma_start(out=ones_t[:], in_=ones_d[:])

            wq_t, wk_t, wv_t, wo_t, bq_t, bk_t, bv4_t, bo_t = [], [], [], [], [], [], [], []
            for p in range(P):
                for lst, dram, shape, tag, dt in (
                    (wq_t, wq_d, [H, H], "wq", BF16),
                    (wk_t, wk_d, [H, H], "wk", BF16),
                    (wv_t, wv_d, [H, H], "wv", BF16),
                    (wo_t, wo_d, [H, H], "wo", BF16),
                    (bq_t, bq_d, [H, 1], "bq", F32),
                    (bk_t, bk_d, [H, 1], "bk", F32),
                    (bv4_t, bv4_d, [1, NCH * H], "bv4", BF16),
                    (bo_t, bo_d, [H, 1], "bo", F32),
                ):
                    t = wpool.tile(shape, dt, tag=f"{tag}{p}")
                    nc.sync.dma_start(out=t[:], in_=dram[p])
                    lst.append(t)

            # ---- per-item pipeline ----
            for i in range(IPC):
                xt_t = xpool.tile([DIN, N], BF16, tag="xt")
                nc.sync.dma_start(out=xt_t[:], in_=xt_d[i])
                mt_t = mpool.tile([128, NCH * N], BF16, tag="mt")
                nc.sync.dma_start(
                    out=mt_t[:], in_=mt_d[i].rearrange("(c p) n -> p c n", c=NCH)
                )

                # encoder: hT = relu(enc_w.T @ xT + enc_b)
                hp = ppsum.tile([H, N], F32, tag="proj")
                nc.tensor.matmul(hp[:], lhsT=(encw_t[:]), rhs=(xt_t[:]), start=True, stop=True)
                hT = hpool.tile([H, N], BF16, tag="h")
                nc.scalar.activation(hT[:], hp[:], AF.Relu, bias=encb_t[:])

                for p in range(P):
                    # projections: q (relu on ACT), k (relu on DVE)
                    qp = ppsum.tile([H, N], F32, tag="proj")
                    nc.tensor.matmul(qp[:], lhsT=(wq_t[p][:]), rhs=(hT[:]), start=True, stop=True)
                    qT = qpool.tile([H, N], BF16, tag="q")
                    nc.scalar.activation(qT[:], qp[:], AF.Relu, bias=bq_t[p][:])

                    kp = ppsum.tile([H, N], F32, tag="proj")
                    nc.tensor.matmul(kp[:], lhsT=(wk_t[p][:]), rhs=(hT[:]), start=True, stop=True)
                    kT = kpool.tile([H, N], BF16, tag="k")
                    nc.vector.tensor_scalar(
                        out=kT[:], in0=kp[:], scalar1=bk_t[p][:], scalar2=0.0,
                        op0=OP.add, op1=OP.max,
                    )

                    # v in natural [m, h] layout: all 4 row chunks in one
                    # [128, 4H] PSUM tile. Bias varies along the FREE axis
                    # here, so preload ones(x)bv4 via a K=1 matmul, then
                    # accumulate the 4 hT-chunk @ wv products on top.
                    vp = ppsum.tile([128, NCH * H], F32, tag="proj")
                    nc.tensor.matmul(
                        vp[:], lhsT=(ones_t[0:1, :]), rhs=(bv4_t[p][:]),
                        start=True, stop=False,
                    )
                    for c in range(NCH):
                        nc.tensor.matmul(
                            vp[:, ts(c, H)], lhsT=(hT[:, ts(c, 128)]), rhs=(wv_t[p][:]),
                            start=False, stop=(c == NCH - 1),
                        )
                    vn = vpool.tile([128, NCH * H], BF16, tag="v")
                    nc.vector.tensor_scalar_max(vn[:], vp[:], 0.0)

                    # scoresT[m, n] in two wide [128, 2N] PSUM tiles; exp is
                    # one wide ACTIVATE per pair -> pT (bf16, SBUF)
                    pT = ppool.tile([128, NCH * N], BF16, tag="p")
                    for j in range(NCH // 2):
                        scp = spsum.tile([128, 2 * N], F32, tag="sc")
                        for cc in range(2):
                            c = 2 * j + cc
                            nc.tensor.matmul(
                                scp[:, ts(cc, N)], lhsT=(kT[:, ts(c, 128)]), rhs=(qT[:]),
                                start=True, stop=True,
                            )
                        nc.scalar.activation(pT[:, ts(j, 2 * N)], scp[:], AF.Exp)
                    # mask multiply (elementwise): 2 chunks DVE, 2 GpSimd
                    for c in range(NCH):
                        eng = nc.vector if c % 2 == 0 else nc.gpsimd
                        eng.tensor_tensor(
                            out=pT[:, ts(c, N)], in0=pT[:, ts(c, N)],
                            in1=mt_t[:, ts(c, N)], op=OP.mult,
                        )

                    # rowsum over m via all-ones matmul (broadcast to all parts)
                    rs = rpsum.tile([128, N], F32, tag="rs")
                    for c in range(NCH):
                        nc.tensor.matmul(
                            rs[:], lhsT=(ones_t[:]), rhs=(pT[:, ts(c, N)]),
                            start=(c == 0), stop=(c == NCH - 1),
                        )
                    # outT = v.T @ p  (accumulate over m chunks)
                    ot = opsum.tile([H, N], F32, tag="ot")
                    for c in range(NCH):
                        nc.tensor.matmul(
                            ot[:], lhsT=(vn[:, ts(c, H)]), rhs=(pT[:, ts(c, N)]),
                            start=(c == 0), stop=(c == NCH - 1),
                        )
                    # 1/rowsum = exp(-ln(rowsum)): Ln and Exp live in the
                    # same ACT table set as the softmax Exp, so no table
                    # switching (Reciprocal would thrash 2.7us loads).
                    lnr = rpool.tile([H, N], F32, tag="lnr")
                    nc.scalar.activation(lnr[:], rs[:], AF.Ln)
                    recipb = rpool.tile([H, N], F32, tag="recip")
                    nc.scalar.activation(recipb[:], lnr[:], AF.Exp, scale=-1.0)
                    otn = opool.tile([H, N], BF16, tag="otn")
                    nc.vector.tensor_tensor(out=otn[:], in0=ot[:], in1=recipb[:], op=OP.mult)

                    # out projection -> new hT
                    h2p = ppsum.tile([H, N], F32, tag="proj")
                    nc.tensor.matmul(h2p[:], lhsT=(wo_t[p][:]), rhs=(otn[:]), start=True, stop=True)
                    hT = hpool.tile([H, N], BF16, tag="h")
                    nc.vector.tensor_scalar(
                        out=hT[:], in0=h2p[:], scalar1=bo_t[p][:], scalar2=0.0,
                        op0=OP.add, op1=OP.max,
                    )

                # Q head: yT = qw.T @ hT + qb
                yp = ppsum.tile([A, N], F32, tag="proj")
                nc.tensor.matmul(yp[:], lhsT=(qw_t[:]), rhs=(hT[:]), start=True, stop=True)
                yt_t = ypool.tile([A, N], F32, tag="y")
                nc.vector.tensor_scalar_add(yt_t[:], yp[:], qb_t[:])
                nc.sync.dma_start(out=yt_d[i], in_=yt_t[:])

    _spill_excess_waits(nc)
    return nc


_prog_cache = None


def _get_program():
    global _prog_cache
    if _prog_cache is None:
        _prog_cache = build_program()
    return _prog_cache


def _make_in_maps(x, mask, enc_w, enc_b, wv, bv, wk, bk, wq, bq, wo, bo, qw, qb):
    import ml_dtypes
    bf = lambda a: np.ascontiguousarray(np.asarray(a, dtype=np.float32).astype(ml_dtypes.bfloat16))
    f = lambda a: np.ascontiguousarray(np.asarray(a, dtype=np.float32))
    x, mask = f(x), f(mask)
    shared = {
        "enc_w": bf(enc_w),
        "enc_b": f(enc_b).reshape(H, 1),
        "wq": bf(wq),
        "wk": bf(wk),
        "wv": bf(wv),
        "wo": bf(wo),
        "bq": f(bq).reshape(P, H, 1),
        "bk": f(bk).reshape(P, H, 1),
        "bv4": np.ascontiguousarray(np.tile(bf(bv), (1, NCH)).reshape(P, 1, NCH * H)),
        "bo": f(bo).reshape(P, H, 1),
        "qw": bf(qw),
        "ones": np.ones((128, 128), dtype=ml_dtypes.bfloat16),
        "qb": f(qb).reshape(A, 1),
    }
    in_maps = []
    for c in range(N_CORES):
        sl = slice(c * IPC, (c + 1) * IPC)
        in_maps.append({
            "xt": np.ascontiguousarray(x[sl].transpose(0, 2, 1).astype(ml_dtypes.bfloat16)),
            "maskt": np.ascontiguousarray(mask[sl].transpose(0, 2, 1).astype(ml_dtypes.bfloat16)),
            **shared,
        })
    return in_maps


def run(trace=False, **inputs):
    nc = _get_program()
    in_maps = _make_in_maps(**inputs)
    res = run_bass_kernel_spmd(nc, in_maps, list(range(N_CORES)), trace=trace)
    y = np.concatenate(
        [r["yt"].transpose(0, 2, 1) for r in res.results], axis=0
    ).astype(np.float32)
    return y, res


def kernel(**inputs):
    y, _ = run(trace=False, **inputs)
    return y
